# revision 1
# baseline (speedup 1.0000x reference)
"""Trainium2 Bass kernel for GTStepwiseConstantVelocityModel.

Strategy: shard node-pair work across 8 cores via a circulant pairing
(node n owns pairs (n, (n+d) mod 384) for d=1..191; pairs at d=192 are a
separate strip split across cores). Each core gets row-rotated copies of
the inputs so the compiled SPMD program is identical on every core.
Layout on device: t (=128) on partitions, node columns on the free axis.
Step positions come from a lower-triangular matmul on the TensorEngine
(cumsum), the elementwise intensity-integral pipeline runs on DVE+ACT
(ln/exp instead of sqrt/rsqrt to stay in one ACT table set), and the
event term is computed with host-built one-hot gather matmuls. Each core
emits [event_partial, nonevent_partial]; the host sums the 8 pairs.
"""
import numpy as np

N, D, T, E, NC = 384, 2, 128, 256, 8
EXTW = 576          # extended (wrapped) column count
G = 4               # rows per chunk in the main loop
NROW = N // NC      # 48 rows per core
EV_PER = E // NC    # 32 events per core
SP_PER = 192 // NC  # 24 strip pairs per core
F = G * 191

_CACHE = {}


def _build_program(dt):
    from contextlib import ExitStack
    import concourse.bacc as bacc
    import concourse.tile as tile
    import concourse.mybir as mybir

    f32 = mybir.dt.float32
    AF = mybir.ActivationFunctionType
    OP = mybir.AluOpType
    AX = mybir.AxisListType
    LN_SPI2 = float(np.log(np.sqrt(np.pi) / 2.0))

    nc = bacc.Bacc("TRN2", target_bir_lowering=False, debug=False, num_devices=NC)

    def din(name, shape):
        return nc.dram_tensor(name, shape, f32, kind="ExternalInput").ap()

    vxe_d = din("vxe", [T, EXTW])
    vye_d = din("vye", [T, EXTW])
    z0xe_d = din("z0xe", [1, EXTW])
    z0ye_d = din("z0ye", [1, EXTW])
    lmat_d = din("lmat", [T, T])
    vstrip_d = din("vstrip", [T, 4 * SP_PER])
    z0strip_d = din("z0strip", [1, 4 * SP_PER])
    vnatx_d = din("vnatx", [N, T])
    vnaty_d = din("vnaty", [N, T])
    qmat_d = din("qmat", [N, EV_PER])
    wmat_d = din("wmat", [T, EV_PER])
    bhot_d = din("bhot", [T, EV_PER])
    dz0x_d = din("dz0x", [1, EV_PER])
    dz0y_d = din("dz0y", [1, EV_PER])
    betac_d = din("betac", [T, 1])
    out_d = nc.dram_tensor("out", [1, 2], f32, kind="ExternalOutput").ap()

    with ExitStack() as ctx:
        tc = ctx.enter_context(tile.TileContext(nc))
        sg = ctx.enter_context(tc.tile_pool(name="singles", bufs=1))
        wk = ctx.enter_context(tc.tile_pool(name="work", bufs=2))
        ps = ctx.enter_context(tc.tile_pool(name="psum", bufs=1, space="PSUM"))

        def load(dram, shape, tag):
            t = sg.tile(shape, f32, tag=tag)
            nc.sync.dma_start(out=t[:], in_=dram[:])
            return t

        vxe = load(vxe_d, [T, EXTW], "vxe")
        vye = load(vye_d, [T, EXTW], "vye")
        z0xe = load(z0xe_d, [1, EXTW], "z0xe")
        z0ye = load(z0ye_d, [1, EXTW], "z0ye")
        lmat = load(lmat_d, [T, T], "lmat")
        vstrip = load(vstrip_d, [T, 4 * SP_PER], "vstrip")
        z0strip = load(z0strip_d, [1, 4 * SP_PER], "z0strip")
        vnx = [sg.tile([128, T], f32, name=f"vnx{r}", tag=f"vnx{r}") for r in range(3)]
        vny = [sg.tile([128, T], f32, name=f"vny{r}", tag=f"vny{r}") for r in range(3)]
        qm = [sg.tile([128, EV_PER], f32, name=f"qm{r}", tag=f"qm{r}") for r in range(3)]
        for r in range(3):
            nc.sync.dma_start(out=vnx[r][:], in_=vnatx_d[128 * r:128 * (r + 1), :])
            nc.sync.dma_start(out=vny[r][:], in_=vnaty_d[128 * r:128 * (r + 1), :])
            nc.sync.dma_start(out=qm[r][:], in_=qmat_d[128 * r:128 * (r + 1), :])
        wmat = load(wmat_d, [T, EV_PER], "wmat")
        bhot = load(bhot_d, [T, EV_PER], "bhot")
        dz0x = load(dz0x_d, [1, EV_PER], "dz0x")
        dz0y = load(dz0y_d, [1, EV_PER], "dz0y")
        betac = load(betac_d, [T, 1], "betac")

        ones = sg.tile([T, 1], f32)
        nc.vector.memset(ones[:], 1.0)
        lones = sg.tile([1, T], f32)
        nc.vector.memset(lones[:], 1.0)
        zcol = sg.tile([T, 1], f32)
        nc.vector.memset(zcol[:], 0.0)
        epscol = sg.tile([T, 1], f32)
        nc.vector.memset(epscol[:], 1e-12)
        bln = sg.tile([T, 1], f32)
        nc.vector.tensor_scalar_add(out=bln[:], in0=betac[:], scalar1=LN_SPI2)

        # ---- step positions ZxE/ZyE via triangular matmul (cumsum + z0) ----
        zxe = sg.tile([T, EXTW], f32)
        zye = sg.tile([T, EXTW], f32)
        for (vsrc, zrow, zdst) in ((vxe, z0xe, zxe), (vye, z0ye, zye)):
            for fc in range(2):
                cs = slice(288 * fc, 288 * (fc + 1))
                pz = ps.tile([T, 288], f32)
                nc.tensor.matmul(pz[:], lmat[0:127, :], vsrc[0:127, cs],
                                 start=True, stop=False)
                nc.tensor.matmul(pz[:], lones[:], zrow[:, cs],
                                 start=False, stop=True)
                nc.scalar.copy(zdst[:, cs], pz[:])

        # strip step positions
        pzs = ps.tile([T, 4 * SP_PER], f32)
        nc.tensor.matmul(pzs[:], lmat[0:127, :], vstrip[0:127, :],
                         start=True, stop=False)
        nc.tensor.matmul(pzs[:], lones[:], z0strip[:],
                         start=False, stop=True)
        zstrip = sg.tile([T, 4 * SP_PER], f32)
        nc.scalar.copy(zstrip[:], pzs[:])

        # ---- event term ----
        gdx = ps.tile([T, EV_PER], f32)
        gdy = ps.tile([T, EV_PER], f32)
        for r in range(3):
            nc.tensor.matmul(gdx[:], vnx[r][:], qm[r][:],
                             start=(r == 0), stop=(r == 2))
        for r in range(3):
            nc.tensor.matmul(gdy[:], vny[r][:], qm[r][:],
                             start=(r == 0), stop=(r == 2))
        hx = wk.tile([T, EV_PER], f32, tag="hx")
        hy = wk.tile([T, EV_PER], f32, tag="hy")
        nc.vector.tensor_mul(hx[:], gdx[:], wmat[:])
        nc.vector.tensor_mul(hy[:], gdy[:], wmat[:])
        shx = ps.tile([1, EV_PER], f32)
        shy = ps.tile([1, EV_PER], f32)
        brow = ps.tile([1, EV_PER], f32)
        nc.tensor.matmul(shx[:], ones[:], hx[:])
        nc.tensor.matmul(shy[:], ones[:], hy[:])
        nc.tensor.matmul(brow[:], betac[:], bhot[:])
        evx = sg.tile([1, EV_PER], f32)
        evy = sg.tile([1, EV_PER], f32)
        nc.vector.tensor_add(evx[:], shx[:], dz0x[:])
        nc.vector.tensor_add(evy[:], shy[:], dz0y[:])
        nc.vector.tensor_mul(evx[:], evx[:], evx[:])
        nc.vector.tensor_mul(evy[:], evy[:], evy[:])
        nc.vector.tensor_add(evx[:], evx[:], evy[:])
        evel = sg.tile([1, EV_PER], f32)
        nc.vector.tensor_sub(evel[:], brow[:], evx[:])
        ev_s = sg.tile([1, 1], f32)
        nc.vector.reduce_sum(out=ev_s[:], in_=evel[:], axis=AX.X)

        # ---- main circulant pipeline ----
        part_cols = []

        def pipeline(fw, dvx, dvy, dzx, dzy):
            s1 = wk.tile([T, F], f32, tag="s1")
            s2 = wk.tile([T, F], f32, tag="s2")
            nc.scalar.activation(s1[:, :fw], dvx, AF.Square, bias=zcol[:])
            nc.scalar.activation(s2[:, :fw], dvy, AF.Square, bias=zcol[:])
            a2 = wk.tile([T, F], f32, tag="a2")
            nc.vector.tensor_add(a2[:, :fw], s1[:, :fw], s2[:, :fw])
            lg = wk.tile([T, F], f32, tag="lg")
            nc.scalar.activation(lg[:, :fw], a2[:, :fw], AF.Ln, bias=epscol[:])
            av = wk.tile([T, F], f32, tag="av")
            nc.scalar.activation(av[:, :fw], lg[:, :fw], AF.Exp, scale=0.5, bias=zcol[:])
            inva = wk.tile([T, F], f32, tag="inva")
            nc.scalar.activation(inva[:, :fw], lg[:, :fw], AF.Exp, scale=-0.5, bias=zcol[:])
            bp = wk.tile([T, F], f32, tag="bp")
            bq = wk.tile([T, F], f32, tag="bq")
            nc.vector.tensor_mul(bp[:, :fw], dzx, dvx)
            nc.vector.tensor_mul(bq[:, :fw], dzy, dvy)
            bv = wk.tile([T, F], f32, tag="bv")
            nc.vector.tensor_add(bv[:, :fw], bp[:, :fw], bq[:, :fw])
            arg2 = wk.tile([T, F], f32, tag="arg2")
            nc.vector.tensor_mul(arg2[:, :fw], bv[:, :fw], inva[:, :fw])
            # r2 (reuse s1/s2/bp)
            nc.scalar.activation(s1[:, :fw], dzx, AF.Square, bias=zcol[:])
            nc.scalar.activation(s2[:, :fw], dzy, AF.Square, bias=zcol[:])
            r2 = wk.tile([T, F], f32, tag="r2")
            nc.vector.tensor_add(r2[:, :fw], s1[:, :fw], s2[:, :fw])
            sqa2 = wk.tile([T, F], f32, tag="sqa2")
            nc.scalar.activation(sqa2[:, :fw], arg2[:, :fw], AF.Square, bias=zcol[:])
            mres = wk.tile([T, F], f32, tag="mres")
            nc.vector.tensor_sub(mres[:, :fw], r2[:, :fw], sqa2[:, :fw])
            arg1 = wk.tile([T, F], f32, tag="arg1")
            nc.vector.scalar_tensor_tensor(
                out=arg1[:, :fw], in0=av[:, :fw], scalar=float(dt),
                in1=arg2[:, :fw], op0=OP.mult, op1=OP.add)
            wv = wk.tile([T, F], f32, tag="wv")
            nc.vector.scalar_tensor_tensor(
                out=wv[:, :fw], in0=lg[:, :fw], scalar=0.5,
                in1=mres[:, :fw], op0=OP.mult, op1=OP.add)
            exiv = wk.tile([T, F], f32, tag="exiv")
            nc.scalar.activation(exiv[:, :fw], wv[:, :fw], AF.Exp,
                                 bias=bln[:], scale=-1.0)
            e1 = wk.tile([T, F], f32, tag="e1")
            e2 = wk.tile([T, F], f32, tag="e2")
            nc.scalar.activation(e1[:, :fw], arg1[:, :fw], AF.Erf, bias=zcol[:])
            nc.scalar.activation(e2[:, :fw], arg2[:, :fw], AF.Erf, bias=zcol[:])
            ed = wk.tile([T, F], f32, tag="ed")
            nc.vector.tensor_sub(ed[:, :fw], e1[:, :fw], e2[:, :fw])
            t4 = wk.tile([T, F], f32, tag="t4")
            nc.vector.tensor_mul(t4[:, :fw], ed[:, :fw], exiv[:, :fw])
            col = sg.tile([T, 1], f32, name=f"col{len(part_cols)}", tag=f"col{len(part_cols)}")
            nc.vector.reduce_sum(out=col[:], in_=t4[:, :fw], axis=AX.X)
            part_cols.append(col)

        for ci in range(NROW // G):
            dvx = wk.tile([T, F], f32, tag="dvx")
            dvy = wk.tile([T, F], f32, tag="dvy")
            dzx = wk.tile([T, F], f32, tag="dzx")
            dzy = wk.tile([T, F], f32, tag="dzy")
            for g in range(G):
                k = ci * G + g
                j0 = 8 * k
                s = slice(191 * g, 191 * (g + 1))
                cs = slice(j0 + 1, j0 + 192)
                nc.vector.tensor_scalar_sub(out=dvx[:, s], in0=vxe[:, cs],
                                            scalar1=vxe[:, j0:j0 + 1])
                nc.vector.tensor_scalar_sub(out=dvy[:, s], in0=vye[:, cs],
                                            scalar1=vye[:, j0:j0 + 1])
                nc.vector.tensor_scalar_sub(out=dzx[:, s], in0=zxe[:, cs],
                                            scalar1=zxe[:, j0:j0 + 1])
                nc.vector.tensor_scalar_sub(out=dzy[:, s], in0=zye[:, cs],
                                            scalar1=zye[:, j0:j0 + 1])
            pipeline(F, dvx[:], dvy[:], dzx[:], dzy[:])

        # strip pipeline (width 24)
        sw = SP_PER
        dvxs = wk.tile([T, sw], f32, tag="dvxs")
        dvys = wk.tile([T, sw], f32, tag="dvys")
        dzxs = wk.tile([T, sw], f32, tag="dzxs")
        dzys = wk.tile([T, sw], f32, tag="dzys")
        nc.vector.tensor_sub(dvxs[:], vstrip[:, 0:sw], vstrip[:, sw:2 * sw])
        nc.vector.tensor_sub(dvys[:], vstrip[:, 2 * sw:3 * sw], vstrip[:, 3 * sw:4 * sw])
        nc.vector.tensor_sub(dzxs[:], zstrip[:, 0:sw], zstrip[:, sw:2 * sw])
        nc.vector.tensor_sub(dzys[:], zstrip[:, 2 * sw:3 * sw], zstrip[:, 3 * sw:4 * sw])
        pipeline(sw, dvxs[:], dvys[:], dzxs[:], dzys[:])

        # ---- reduce partials and write out ----
        while len(part_cols) > 1:
            nxt = []
            for i in range(0, len(part_cols) - 1, 2):
                dst = sg.tile([T, 1], f32, name=f"red{len(nxt)}_{len(part_cols)}", tag=f"red{len(nxt)}_{len(part_cols)}")
                nc.vector.tensor_add(dst[:], part_cols[i][:], part_cols[i + 1][:])
                nxt.append(dst)
            if len(part_cols) % 2:
                nxt.append(part_cols[-1])
            part_cols = nxt
        s_ps = ps.tile([1, 1], f32)
        nc.tensor.matmul(s_ps[:], part_cols[0][:], ones[:])
        out_sb = sg.tile([1, 2], f32)
        nc.vector.tensor_copy(out_sb[:, 0:1], ev_s[:])
        nc.vector.tensor_copy(out_sb[:, 1:2], s_ps[:])
        nc.sync.dma_start(out=out_d[:], in_=out_sb[:])

    nc.finalize()
    return nc


def _host_prep(data, t0, tn, z0, v0, beta):
    dt = float(tn - t0) / T
    v0x, v0y = np.ascontiguousarray(v0[:, 0, :]), np.ascontiguousarray(v0[:, 1, :])
    z0x, z0y = z0[:, 0], z0[:, 1]

    lmat = np.zeros((T, T), np.float32)
    for k in range(T - 1):
        lmat[k, k + 1:] = dt
    lmat[T - 1, :] = 1.0

    times = data[:, 2]
    idx_f = np.floor(times / dt)
    idx = np.where(idx_f < T, idx_f, idx_f - 1.0).astype(np.int32)
    rem = (times - idx_f * dt).astype(np.float32)
    i_idx = np.floor(data[:, 0]).astype(np.int32)
    j_idx = np.floor(data[:, 1]).astype(np.int32)

    in_maps = []
    for c in range(NC):
        ridx = (np.arange(EXTW) + c) % N
        m = {
            "vxe": np.ascontiguousarray(v0x[ridx, :].T),
            "vye": np.ascontiguousarray(v0y[ridx, :].T),
            "z0xe": np.ascontiguousarray(z0x[ridx][None, :]),
            "z0ye": np.ascontiguousarray(z0y[ridx][None, :]),
            "lmat": lmat,
            "vnatx": v0x, "vnaty": v0y,
            "betac": np.ascontiguousarray(beta[:, None]),
        }
        sA = np.arange(SP_PER * c, SP_PER * (c + 1))
        sB = sA + 192
        m["vstrip"] = np.ascontiguousarray(
            np.concatenate([v0x[sA].T, v0x[sB].T, v0y[sA].T, v0y[sB].T], axis=1))
        m["z0strip"] = np.concatenate(
            [z0x[sA], z0x[sB], z0y[sA], z0y[sB]])[None, :].astype(np.float32)
        es = slice(EV_PER * c, EV_PER * (c + 1))
        ii, jj, dd, rr = i_idx[es], j_idx[es], idx[es], rem[es]
        Q = np.zeros((N, EV_PER), np.float32)
        W = np.zeros((T, EV_PER), np.float32)
        B = np.zeros((T, EV_PER), np.float32)
        for e in range(EV_PER):
            Q[ii[e], e] += 1.0
            Q[jj[e], e] -= 1.0
            W[:dd[e], e] = dt
            W[dd[e], e] += rr[e]
            B[dd[e], e] = 1.0
        m["qmat"], m["wmat"], m["bhot"] = Q, W, B
        m["dz0x"] = (z0x[ii] - z0x[jj])[None, :].astype(np.float32)
        m["dz0y"] = (z0y[ii] - z0y[jj])[None, :].astype(np.float32)
        in_maps.append({k: np.ascontiguousarray(v, dtype=np.float32)
                        for k, v in m.items()})
    return dt, in_maps


def _run(inputs, trace=False):
    from concourse.bass_utils import run_bass_kernel_spmd
    data = np.asarray(inputs["data"], np.float32)
    t0 = float(np.asarray(inputs["t0"]))
    tn = float(np.asarray(inputs["tn"]))
    z0 = np.asarray(inputs["z0"], np.float32)
    v0 = np.asarray(inputs["v0"], np.float32)
    beta = np.asarray(inputs["beta"], np.float32)

    dt, in_maps = _host_prep(data, t0, tn, z0, v0, beta)
    if dt not in _CACHE:
        _CACHE[dt] = _build_program(dt)
    nc = _CACHE[dt]
    res = run_bass_kernel_spmd(nc, in_maps, core_ids=list(range(NC)), trace=trace)
    ev = sum(float(res.results[c]["out"][0, 0]) for c in range(NC))
    S = sum(float(res.results[c]["out"][0, 1]) for c in range(NC))
    return np.array(np.float32(ev - S)), res


def kernel(**inputs):
    out, _ = _run(inputs, trace=False)
    return out

